# revision 1
# baseline (speedup 1.0000x reference)
"""Trainium2 Bass kernel for CAM (channel attention module).

Reference computation (per batch b):
    q = x_low[b]  as [C, N]   (C=512, N=64*64=4096)
    k = x_high[b] as [C, N]
    E = q @ k.T                              # [C, C]
    att = softmax(rowmax(E) - E, axis=-1)    # == exp(rowmin(E) - E) / Z
    out = gamma * (att @ k) + x_low[b]

Sharding: data-parallel over batch. 16 batches / 8 cores = 2 per core.
gamma is replicated (pre-broadcast on host to [128,1]).

Matmuls run in float32r (full-rate fp32 PE mode). The n-major operand
layouts needed by the first matmul (contraction over N) are produced with
PE transposes (128x128 blocks via identity matmul) whose PSUM results are
copied to SBUF on DVE (q) and ACT (k) to balance engine load.

Inputs are loaded as [128, 1024] sub-tiles in n-major issue order so the
transpose+mm1 pipeline starts after the first ~4 MiB arrives, and mm2
iterates n-blocks outermost so input slots free early, letting the next
batch's loads overlap the current batch's tail compute.
"""

import sys

sys.path.insert(0, "/opt/trn_rl_repo")

import numpy as np

B, C, H, W = 16, 512, 64, 64
N = H * W               # 4096
N_CORES = 8
B_LOC = B // N_CORES    # 2 batches per core
P = 128                 # partitions
CP = C // P             # 4 channel chunks
NP = N // P             # 32 n chunks of 128
FB = 512                # free-dim block (psum bank) for mm2 output
NB = N // FB            # 8 n blocks of 512
ST = 512                # load sub-tile free size
NS = N // ST            # 4 sub-tiles per (tensor, cc)

RESIDUAL_VIA_PE = False   # fold +x_low into mm2 as identity matmul, epilogue on ACT
MM2_PSUM_FROM_E = False    # mm2 accumulators reuse the (dead) E banks
MODE = "f32r"             # "fp16" | "f32r"

_CACHE = {}


def _build_module(reps=0, variant="full", st=None):
    global ST, NS
    if st is not None:
        ST, NS = st, N // st
    import concourse.bacc as bacc
    import concourse.tile as tile
    import concourse.mybir as mybir
    from concourse.masks import make_identity

    f32 = mybir.dt.float32
    f32r = mybir.dt.float32r

    nc = bacc.Bacc("TRN2", target_bir_lowering=False, debug=False)

    xh = nc.dram_tensor("xh", [B_LOC, C, N], f32r, kind="ExternalInput")
    xl = nc.dram_tensor("xl", [B_LOC, C, N], f32r, kind="ExternalInput")
    gm = nc.dram_tensor("gm", [P, 1], f32, kind="ExternalInput")
    out = nc.dram_tensor("out", [B_LOC, C, N], f32, kind="ExternalOutput")

    def r(ap):
        return ap.bitcast(f32r)

    def rf(ap):
        return ap.bitcast(f32)

    with tile.TileContext(nc) as tc:
        with (
            tc.tile_pool(name="const", bufs=1) as const_pool,
            tc.tile_pool(name="kn", bufs=NS * CP + max(1, 8 // (ST // 512))) as kn_pool,
            tc.tile_pool(name="qn", bufs=NS * CP + max(1, 8 // (ST // 512))) as qn_pool,
            tc.tile_pool(name="tT", bufs=2) as tT_pool,
            tc.tile_pool(name="soft", bufs=CP) as soft_pool,
            tc.tile_pool(name="attT", bufs=CP) as attT_pool,
            tc.tile_pool(name="osb", bufs=8) as out_pool,
            tc.tile_pool(name="small", bufs=24) as small_pool,
            tc.tile_pool(name="psE", bufs=CP, space="PSUM") as psE_pool,
            tc.tile_pool(name="psW", bufs=4, space="PSUM") as psW_pool,
        ):
            ident_f = const_pool.tile([P, P], f32)
            make_identity(nc, ident_f[:])
            ident = const_pool.tile([P, P], f32r)
            nc.vector.tensor_copy(ident[:], ident_f[:])
            gsb = const_pool.tile([P, 1], f32)
            nc.sync.dma_start(gsb[:], gm.ap())

            import contextlib
            rep_ctx = tc.For_i(0, reps, 1) if reps else contextlib.nullcontext()
            with rep_ctx:
              for b in range(B_LOC):
                # ---- load natural tiles as sub-tiles, n-major issue order ----
                KN = [[None] * NS for _ in range(CP)]
                QN = [[None] * NS for _ in range(CP)]
                for s in range(NS):
                    ssl = slice(s * ST, (s + 1) * ST)
                    for cc in range(CP):
                        csl = slice(cc * P, (cc + 1) * P)
                        kt = kn_pool.tile([P, ST], f32r, tag="kn", name=f"kn{b}_{cc}_{s}")
                        qt = qn_pool.tile([P, ST], f32r, tag="qn", name=f"qn{b}_{cc}_{s}")
                        if variant == "noload":
                            nc.sync.dma_start(kt[:, 0:4], xh.ap()[b, csl, s * ST:s * ST + 4])
                            nc.sync.dma_start(qt[:, 0:4], xl.ap()[b, csl, s * ST:s * ST + 4])
                        else:
                            nc.sync.dma_start(kt[:], xh.ap()[b, csl, ssl])
                            nc.sync.dma_start(qt[:], xl.ap()[b, csl, ssl])
                        KN[cc][s] = kt
                        QN[cc][s] = qt

                def qn_blk(cc, lo, width):
                    s = lo // ST
                    o = lo - s * ST
                    return QN[cc][s][:, o:o + width]

                def kn_blk(cc, lo, width):
                    s = lo // ST
                    o = lo - s * ST
                    return KN[cc][s][:, o:o + width]

                if variant == "dma":
                    for nb in range(NB):
                        for ic in range(CP):
                            isl = slice(ic * P, (ic + 1) * P)
                            src = qn_blk(ic, nb * FB, FB) if nb % 2 == 0 else kn_blk(ic, nb * FB, FB)
                            nc.scalar.dma_start(out.ap()[b, isl, nb * FB:(nb + 1) * FB],
                                                rf(src))
                    continue

                # ---- transposes + mm1: E[i,j] accumulated over n chunks ----
                E = [psE_pool.tile([P, FB], f32, tag="E", name=f"E{b}_{i}") for i in range(CP)]
                for nn in range(NP):
                    qtp = psW_pool.tile([P, FB], f32, tag="wp")
                    ktp = psW_pool.tile([P, FB], f32, tag="wp")
                    for cc in range(CP):
                        csl = slice(cc * P, (cc + 1) * P)
                        nc.tensor.transpose(
                            r(qtp[:, csl]), r(qn_blk(cc, nn * P, P)), r(ident[:]))
                        nc.tensor.transpose(
                            r(ktp[:, csl]), r(kn_blk(cc, nn * P, P)), r(ident[:]))
                    QT = tT_pool.tile([P, FB], f32r, tag="qt")
                    nc.vector.tensor_copy(QT[:], qtp[:])
                    KT = tT_pool.tile([P, FB], f32r, tag="kt")
                    nc.scalar.copy(KT[:], ktp[:])
                    for ic in range(CP):
                        nc.tensor.matmul(
                            E[ic][:],
                            r(QT[:, ic * P:(ic + 1) * P]),
                            r(KT[:]),
                            start=(nn == 0),
                            stop=(nn == NP - 1),
                        )

                # ---- softmax (inverted): att = gamma * exp(m - E) / Z ----
                att = []
                for ic in range(CP):
                    m = small_pool.tile([P, 1], f32, tag="m")
                    nc.vector.tensor_reduce(
                        m[:], E[ic][:], axis=mybir.AxisListType.X,
                        op=mybir.AluOpType.min,
                    )
                    a = soft_pool.tile([P, FB], f32r, tag="att")
                    z = small_pool.tile([P, 1], f32, tag="z")
                    nc.scalar.activation(
                        a[:], E[ic][:], mybir.ActivationFunctionType.Exp,
                        bias=m[:], scale=-1.0, accum_out=z[:],
                    )
                    zinv = small_pool.tile([P, 1], f32, tag="zi")
                    nc.vector.reciprocal(zinv[:], z[:])
                    asc = small_pool.tile([P, 1], f32, tag="as")
                    nc.vector.tensor_mul(asc[:], zinv[:], gsb[:])
                    nc.vector.tensor_scalar_mul(a[:], a[:], asc[:])
                    att.append(a)

                if variant == "p1":
                    for ic in range(CP):
                        nc.scalar.dma_start(out.ap()[b, ic * P:(ic + 1) * P, 0:4],
                                            rf(att[ic][:, 0:4]))
                    continue

                # ---- transpose att -> attT[j, i] ----
                attT = []
                for jc in range(CP):
                    atp = psW_pool.tile([P, FB], f32, tag="wp")
                    jsl = slice(jc * P, (jc + 1) * P)
                    for ic in range(CP):
                        nc.tensor.transpose(
                            r(atp[:, ic * P:(ic + 1) * P]), r(att[ic][:, jsl]), r(ident[:])
                        )
                    aT = attT_pool.tile([P, FB], f32r, tag="attT")
                    if jc % 2 == 0:
                        nc.vector.tensor_copy(aT[:], atp[:])
                    else:
                        nc.scalar.copy(aT[:], atp[:])
                    attT.append(aT)

                # ---- mm2 + residual + store (n-blocks outermost) ----
                for nb in range(NB):
                    for ic in range(CP):
                        isl = slice(ic * P, (ic + 1) * P)
                        par = (nb * CP + ic) % 2
                        ops_pool = psE_pool if par == 0 else psW_pool
                        ops_tag = "E" if par == 0 else "wp"
                        ops = ops_pool.tile([P, FB], f32, tag=ops_tag, name=f"ops{b}_{nb}_{ic}")
                        for jc in range(CP):
                            nc.tensor.matmul(
                                ops[:],
                                r(attT[jc][:, isl]),
                                r(kn_blk(jc, nb * FB, FB)),
                                start=(jc == 0),
                                stop=(jc == CP - 1) and not RESIDUAL_VIA_PE,
                            )
                        if RESIDUAL_VIA_PE:
                            nc.tensor.matmul(
                                ops[:],
                                r(ident[:]),
                                r(qn_blk(ic, nb * FB, FB)),
                                start=False,
                                stop=True,
                            )
                        osb = out_pool.tile([P, FB], f32, tag="osb")
                        if RESIDUAL_VIA_PE:
                            nc.scalar.copy(osb[:], ops[:])
                        else:
                            nc.vector.tensor_add(osb[:], ops[:], rf(qn_blk(ic, nb * FB, FB)))
                        nc.scalar.dma_start(out.ap()[b, isl, nb * FB:(nb + 1) * FB], osb[:])

    nc.compile()
    return nc


def _build_fp16(reps=0, variant="full", st=None):
    """fp16 pipeline: DMA-cast loads, fp16 transposes/matmuls, residual via
    identity matmul, fp32 PSUM accumulation throughout."""
    global ST, NS
    if st is not None:
        ST, NS = st, N // st
    import contextlib
    import concourse.bacc as bacc
    import concourse.tile as tile
    import concourse.mybir as mybir
    from concourse.masks import make_identity

    f32 = mybir.dt.float32
    f16 = mybir.dt.float16

    nc = bacc.Bacc("TRN2", target_bir_lowering=False, debug=False)

    xh = nc.dram_tensor("xh", [B_LOC, C, N], f32, kind="ExternalInput")
    xl = nc.dram_tensor("xl", [B_LOC, C, N], f32, kind="ExternalInput")
    gm = nc.dram_tensor("gm", [P, 1], f32, kind="ExternalInput")
    out = nc.dram_tensor("out", [B_LOC, C, N], f32, kind="ExternalOutput")

    with tile.TileContext(nc) as tc:
        with (
            tc.tile_pool(name="const", bufs=1) as const_pool,
            tc.tile_pool(name="kn", bufs=2 * NS * CP) as kn_pool,
            tc.tile_pool(name="qn", bufs=2 * NS * CP) as qn_pool,
            tc.tile_pool(name="tT", bufs=3) as tT_pool,
            tc.tile_pool(name="soft", bufs=CP) as soft_pool,
            tc.tile_pool(name="attT", bufs=CP) as attT_pool,
            tc.tile_pool(name="osb", bufs=10) as out_pool,
            tc.tile_pool(name="small", bufs=24) as small_pool,
            tc.tile_pool(name="psE", bufs=CP, space="PSUM") as psE_pool,
            tc.tile_pool(name="psW", bufs=4, space="PSUM") as psW_pool,
        ):
            ident_f = const_pool.tile([P, P], f32)
            make_identity(nc, ident_f[:])
            identh = const_pool.tile([P, P], f16)
            nc.vector.tensor_copy(identh[:], ident_f[:])
            gsb = const_pool.tile([P, 1], f32)
            nc.sync.dma_start(gsb[:], gm.ap())

            rep_ctx = tc.For_i(0, reps, 1) if reps else contextlib.nullcontext()
            with rep_ctx:
              for b in range(B_LOC):
                KN = [[None] * NS for _ in range(CP)]
                QN = [[None] * NS for _ in range(CP)]
                for s in range(NS):
                    ssl = slice(s * ST, (s + 1) * ST)
                    for cc in range(CP):
                        csl = slice(cc * P, (cc + 1) * P)
                        kt = kn_pool.tile([P, ST], f16, tag="kn", name=f"kn{b}_{cc}_{s}")
                        qt = qn_pool.tile([P, ST], f16, tag="qn", name=f"qn{b}_{cc}_{s}")
                        if variant == "noload":
                            nc.gpsimd.dma_start(kt[:, 0:4], xh.ap()[b, csl, s * ST:s * ST + 4])
                            nc.gpsimd.dma_start(qt[:, 0:4], xl.ap()[b, csl, s * ST:s * ST + 4])
                        else:
                            nc.gpsimd.dma_start(kt[:], xh.ap()[b, csl, ssl])
                            nc.gpsimd.dma_start(qt[:], xl.ap()[b, csl, ssl])
                        KN[cc][s] = kt
                        QN[cc][s] = qt

                def qn_blk(cc, lo, width):
                    s = lo // ST
                    o = lo - s * ST
                    return QN[cc][s][:, o:o + width]

                def kn_blk(cc, lo, width):
                    s = lo // ST
                    o = lo - s * ST
                    return KN[cc][s][:, o:o + width]

                if variant == "dma":
                    for nb in range(NB):
                        for ic in range(CP):
                            isl = slice(ic * P, (ic + 1) * P)
                            src = qn_blk(ic, nb * FB, FB) if nb % 2 == 0 else kn_blk(ic, nb * FB, FB)
                            osb = out_pool.tile([P, FB], f32, tag="osb")
                            nc.vector.tensor_copy(osb[:], src)
                            nc.scalar.dma_start(out.ap()[b, isl, nb * FB:(nb + 1) * FB], osb[:])
                    continue

                # ---- transposes + mm1 ----
                E = [psE_pool.tile([P, FB], f32, tag="E", name=f"E{b}_{i}") for i in range(CP)]
                for nn in range(NP):
                    qtp = psW_pool.tile([P, FB], f16, tag="wp")
                    ktp = psW_pool.tile([P, FB], f16, tag="wp")
                    for cc in range(CP):
                        csl = slice(cc * P, (cc + 1) * P)
                        nc.tensor.transpose(qtp[:, csl], qn_blk(cc, nn * P, P), identh[:])
                        nc.tensor.transpose(ktp[:, csl], kn_blk(cc, nn * P, P), identh[:])
                    QT = tT_pool.tile([P, FB], f16, tag="qt")
                    nc.vector.tensor_copy(QT[:], qtp[:])
                    KT = tT_pool.tile([P, FB], f16, tag="kt")
                    nc.scalar.copy(KT[:], ktp[:])
                    for ic in range(CP):
                        nc.tensor.matmul(
                            E[ic][:],
                            QT[:, ic * P:(ic + 1) * P],
                            KT[:],
                            start=(nn == 0),
                            stop=(nn == NP - 1),
                        )

                # ---- softmax: att = gamma * exp(m - E) / Z  (fp16 att) ----
                att = []
                for ic in range(CP):
                    m = small_pool.tile([P, 1], f32, tag="m")
                    nc.vector.tensor_reduce(
                        m[:], E[ic][:], axis=mybir.AxisListType.X,
                        op=mybir.AluOpType.min,
                    )
                    a = soft_pool.tile([P, FB], f16, tag="att")
                    z = small_pool.tile([P, 1], f32, tag="z")
                    nc.scalar.activation(
                        a[:], E[ic][:], mybir.ActivationFunctionType.Exp,
                        bias=m[:], scale=-1.0, accum_out=z[:],
                    )
                    zinv = small_pool.tile([P, 1], f32, tag="zi")
                    nc.vector.reciprocal(zinv[:], z[:])
                    asc = small_pool.tile([P, 1], f32, tag="as")
                    nc.vector.tensor_mul(asc[:], zinv[:], gsb[:])
                    nc.vector.tensor_scalar_mul(a[:], a[:], asc[:])
                    att.append(a)

                if variant == "p1":
                    for ic in range(CP):
                        osb = out_pool.tile([P, 4], f32, tag="osb4")
                        nc.vector.tensor_copy(osb[:], att[ic][:, 0:4])
                        nc.scalar.dma_start(out.ap()[b, ic * P:(ic + 1) * P, 0:4], osb[:])
                    continue

                # ---- attT ----
                attT = []
                for jc in range(CP):
                    atp = psW_pool.tile([P, FB], f16, tag="wp")
                    jsl = slice(jc * P, (jc + 1) * P)
                    for ic in range(CP):
                        nc.tensor.transpose(
                            atp[:, ic * P:(ic + 1) * P], att[ic][:, jsl], identh[:]
                        )
                    aT = attT_pool.tile([P, FB], f16, tag="attT")
                    if jc % 2 == 0:
                        nc.vector.tensor_copy(aT[:], atp[:])
                    else:
                        nc.scalar.copy(aT[:], atp[:])
                    attT.append(aT)

                # ---- mm2 + residual(identity matmul) + store ----
                for nb in range(NB):
                    for ic in range(CP):
                        isl = slice(ic * P, (ic + 1) * P)
                        par = (nb * CP + ic) % 2
                        ops_pool = psE_pool if par == 0 else psW_pool
                        ops_tag = "E" if par == 0 else "wp"
                        ops = ops_pool.tile([P, FB], f32, tag=ops_tag, name=f"ops{b}_{nb}_{ic}")
                        for jc in range(CP):
                            nc.tensor.matmul(
                                ops[:],
                                attT[jc][:, isl],
                                kn_blk(jc, nb * FB, FB),
                                start=(jc == 0),
                                stop=False,
                            )
                        nc.tensor.matmul(
                            ops[:],
                            identh[:],
                            qn_blk(ic, nb * FB, FB),
                            start=False,
                            stop=True,
                        )
                        osb = out_pool.tile([P, FB], f32, tag="osb")
                        if (nb * CP + ic) % 2 == 0:
                            nc.scalar.copy(osb[:], ops[:])
                        else:
                            nc.vector.tensor_copy(osb[:], ops[:])
                        nc.sync.dma_start(out.ap()[b, isl, nb * FB:(nb + 1) * FB], osb[:])

    nc.compile()
    return nc


def _build(reps=0, variant="full", st=None, mode=None):
    mode = mode or MODE
    if mode == "fp16":
        return _build_fp16(reps=reps, variant=variant, st=st)
    return _build_module(reps=reps, variant=variant, st=st)


def _get_module():
    if "nc" not in _CACHE:
        _CACHE["nc"] = _build()
    return _CACHE["nc"]


def kernel(x_high, x_low, gamma):
    from concourse.bass_utils import run_bass_kernel_spmd

    nc = _get_module()

    x_high = np.ascontiguousarray(np.asarray(x_high), dtype=np.float32)
    x_low = np.ascontiguousarray(np.asarray(x_low), dtype=np.float32)
    gamma = np.asarray(gamma, dtype=np.float32).reshape(-1)

    xh3 = x_high.reshape(B, C, N)
    xl3 = x_low.reshape(B, C, N)
    gm = np.full((P, 1), gamma[0], dtype=np.float32)

    in_maps = []
    for i in range(N_CORES):
        sl = slice(i * B_LOC, (i + 1) * B_LOC)
        in_maps.append({
            "xh": np.ascontiguousarray(xh3[sl]),
            "xl": np.ascontiguousarray(xl3[sl]),
            "gm": gm,
        })

    res = run_bass_kernel_spmd(nc, in_maps, list(range(N_CORES)))
    out = np.concatenate([res.results[i]["out"] for i in range(N_CORES)], axis=0)
    return out.reshape(B, C, H, W)

